# revision 31
# baseline (speedup 1.0000x reference)
"""A3TGCN (attention temporal GCN) on 8 Trainium2 NeuronCores.

Strategy
--------
The reference computes, per timestep t, three GCN convolutions of the form
segment_sum(norm * (x_t @ W)[src]) which commute with the dense projection:
  A_hat @ (x_t @ W) == (A_hat @ x_t) @ W.
All three convs at every timestep therefore share ONE sparse aggregation
Y = A_hat @ X with X = concat_t x_t  (50k x 192), after which the whole
GRU recurrence + attention is purely node-local dense compute.

Sharding: nodes are partitioned across the 8 cores on the dst axis
(6250 real + 22 pad = 6272 per core). Inputs (X, weights) are replicated,
so no halo exchange is needed at runtime; each core gathers the source
rows for its in-edges directly from HBM via dma_gather.

Host-side one-time preprocessing (graph/weight setup, as in any deployed
GNN): GCN normalization w' = dinv[src]*w*dinv[dst] (the reference marks
this "computed once"), GRU weight folding
  Wz1 = conv_z_w @ lin_z_w[:H],  bz = conv_z_b @ lin_z_w[:H] + lin_z_b
(ditto r/h), and edge bucketing/padding into the fixed device layout.
All O(E*F*T) work — aggregation, GRU, attention, output projection —
runs on device.

Device pipeline per core:
  phase B, per 128-dst block: dma_gather source rows (fp16, 512B rows,
    int16 indices over two 25088-row segments of X, idx groups padded to
    128 so every SBUF chunk is fully overwritten — no memsets, no NaNs).
    Gathers run in PREPARE_ONLY mode: GpSimd only generates descriptors
    and trigger_dma fires them on the 16 DMA engines, so desc-gen for
    block b+1 overlaps the transfer for block b instead of serializing.
    The scatter matrix S_w[e, dmod] = w'_e builds on DVE (broadcast
    is_equal * w); Y_block = S_w^T @ Xg accumulates in PSUM over edge
    chunks, then 48-feature groups transpose so timestep trios sit at
    partition offsets 0/64.
  phase C, per 512-node tile: 12 GRU steps (fp16 matmuls, f32 PSUM).
    sigmoid(x) = (1+tanh(x/2))/2 keeps every ACT function (tanh/relu/
    copy/exp) in ONE activation table. scalar_tensor_tensor fuses the
    GRU update: urh2=(u_r+1)*h gives Wh2*(r*h) in ONE matmul on the
    halved Wh2 slot, and h' = h~ + 0.5*(u_z+1)*(h-h~) needs no separate
    z tile. Attention scores accumulate into a single [12,NT] PSUM tile
    via per-step masked-lhsT matmuls (lhsT column t = att2), so exp runs
    ONCE per tile over [12,NT]; the softmax denominator reduces on
    GpSimd (partition_all_reduce) and the exp rows broadcast back on
    GpSimd (partition_broadcast) for the DVE weighted-h MAC. The
    constant att2 bias cancels in softmax and is dropped; 1/den uses
    the fast DVE reciprocal (|err| ~1e-5 rel, gate is 2e-2).
"""

import numpy as np

N, F, T, H, OUT = 50000, 16, 12, 128, 16
NCORE = 8
PERCORE = N // NCORE            # 6250 real nodes per core
BLK = 128
NBLK = 49                       # 49 blocks of 128 = 6272 padded nodes/core
SEG = 25088                     # X segment rows (int16-indexable)
ROWE = 256                      # fp16 elements per X row (192 data + pad)
FD = F * T                      # 192
P = 128
NT = 512                        # node-tile width in phase C
NNT = 13                        # node tiles per core (13*512 = 6656 >= 6272)
YW = NNT * NT                   # padded out width
USE_GPB = True                  # softmax denominator reduce on GpSimd
XGB = 6                         # xg gather buffers in flight


def _host_prep(x, edge_index, edge_weight):
    """Graph normalization + device data layout. Returns per-core arrays."""
    src = np.concatenate([edge_index[0].astype(np.int64), np.arange(N)])
    dst = np.concatenate([edge_index[1].astype(np.int64), np.arange(N)])
    w = np.concatenate([edge_weight.astype(np.float32), np.ones(N, np.float32)])

    deg = np.bincount(dst, weights=w, minlength=N).astype(np.float32)
    dinv = np.where(deg > 0, 1.0 / np.sqrt(deg), 0.0).astype(np.float32)
    wn = (dinv[src] * w * dinv[dst]).astype(np.float32)

    core = dst // PERCORE
    rloc = dst % PERCORE
    blk = rloc // BLK
    dmod = (rloc % BLK).astype(np.float16)
    seg = (src >= SEG).astype(np.int64)
    idx16 = (src - SEG * seg).astype(np.int16)

    key = (core * NBLK + blk) * 2 + seg
    order = np.argsort(key, kind="stable")
    key_s = key[order]
    cnt = np.bincount(key, minlength=NCORE * NBLK * 2).reshape(NCORE, NBLK, 2)
    # chunks per (block, segment): max over cores so the SPMD graph is shared
    maxc = cnt.max(0)                                          # [NBLK, 2]
    KLO = np.maximum(1, -(-maxc[:, 0] // P))
    KHI = np.maximum(1, -(-maxc[:, 1] // P))
    CK = KLO + KHI
    ck0 = np.zeros(NBLK + 1, np.int64)
    ck0[1:] = np.cumsum(CK)

    # position of each edge within its (core, blk, seg) group
    gstart = np.zeros(NCORE * NBLK * 2 + 1, np.int64)
    gstart[1:] = np.cumsum(cnt.ravel())
    j = np.arange(len(order)) - gstart[key_s]

    core_s = core[order]
    blk_s = blk[order]
    seg_s = seg[order]
    idx_s = idx16[order]
    dmod_s = dmod[order]
    wn_s = wn[order].astype(np.float16)

    totck = int(ck0[-1])
    # global chunk column and lane for the (dmod, w) slot layout
    gchunk = ck0[blk_s] + KLO[blk_s] * seg_s + j // P
    lane = j % P
    # wrapped idx layout: within a gather group, index k sits at
    # [k%16, k//16]; gather for (blk b, seg s) owns cols [8*(ck0+KLO*s)...)
    icol = 8 * (ck0[blk_s] + KLO[blk_s] * seg_s) + j // 16
    irow = j % 16

    dm_arr, wv_arr, ix_arr = [], [], []
    for c in range(NCORE):
        m = core_s == c
        dmc = np.zeros((P, totck), np.float16)
        wvc = np.zeros((P, totck), np.float16)
        dmc[lane[m], gchunk[m]] = dmod_s[m]
        wvc[lane[m], gchunk[m]] = wn_s[m]
        # idx arrays default to 0: every lane of every chunk gathers a
        # real row (row 0 for pads), so chunks are always fully written
        ixc = np.zeros((16, totck * 8), np.int16)
        ixc[irow[m], icol[m]] = idx_s[m]

        dm_arr.append(dmc)
        wv_arr.append(wvc)
        ix_arr.append(np.tile(ixc, (8, 1)))

    # X replica: row n = x[n] laid out t-major (col = t*F + f), fp16, padded
    xt = np.zeros((2 * SEG, ROWE), np.float16)
    xt[:N, :FD] = np.moveaxis(x, 2, 1).reshape(N, FD)
    return dm_arr, wv_arr, ix_arr, xt[:SEG], xt[SEG:], KLO, KHI, CK, ck0


def _fold_weights(inp):
    """Fold conv into lin weights (the concat trick) and build the padded
    timestep-trio lhsT variants used by phase C."""
    f32 = np.float32
    wz1 = inp["conv_z_w"].astype(f32) @ inp["lin_z_w"].astype(f32)[:H]
    wr1 = inp["conv_r_w"].astype(f32) @ inp["lin_r_w"].astype(f32)[:H]
    wh1 = inp["conv_h_w"].astype(f32) @ inp["lin_h_w"].astype(f32)[:H]
    bz = inp["conv_z_b"].astype(f32) @ inp["lin_z_w"].astype(f32)[:H] + inp["lin_z_b"].astype(f32)
    br = inp["conv_r_b"].astype(f32) @ inp["lin_r_w"].astype(f32)[:H] + inp["lin_r_b"].astype(f32)
    bh = inp["conv_h_b"].astype(f32) @ inp["lin_h_w"].astype(f32)[:H] + inp["lin_h_b"].astype(f32)

    # wp[:, (gate*3+v)*128 : +128]: rows [16v:16v+16] and [64+16v:+16] = W
    wp = np.zeros((P, 9 * P), np.float16)
    for g, w1 in enumerate((wz1, wr1, wh1)):
        for v in range(3):
            col = (g * 3 + v) * P
            wp[16 * v:16 * v + 16, col:col + P] = w1.astype(np.float16)
            wp[64 + 16 * v:64 + 16 * v + 16, col:col + P] = w1.astype(np.float16)
    # gate-h slot halved: Wh2*(r*h) is computed as (Wh2/2) @ ((u_r+1)*h)
    w2 = np.concatenate([inp["lin_z_w"][H:], inp["lin_r_w"][H:],
                         inp["lin_h_w"][H:] * 0.5], axis=1).astype(np.float16)
    # per-step score lhsT: block t has att2 in column t, zeros elsewhere,
    # so 12 accumulating matmuls build the [12, NT] score tile in place
    a2w = np.zeros((P, 12 * T), np.float16)
    for t in range(T):
        a2w[:, t * T + t] = inp["att2_w"].astype(np.float16).reshape(H)
    return dict(
        wp=wp, w2=w2,
        att1w=inp["att1_w"].astype(np.float16),
        a2w=a2w,
        outw=inp["out_w"].astype(np.float16),
        bz=(bz / 2).reshape(H, 1).astype(f32),
        br=(br / 2).reshape(H, 1).astype(f32),
        bh=bh.reshape(H, 1).astype(f32),
        ab1=inp["att1_b"].astype(f32).reshape(H, 1),
        outb=inp["out_b"].astype(f32).reshape(OUT, 1),
    )


def _build_graph(totck, KLO, KHI, CK, ck0):
    from concourse import bass, bacc, mybir
    import concourse.tile as tile
    import concourse.bass_isa as bass_isa

    fp16 = mybir.dt.float16
    f32 = mybir.dt.float32
    AF = mybir.ActivationFunctionType
    OP = mybir.AluOpType

    nc = bacc.Bacc("TRN2", target_bir_lowering=False, debug=False,
                   num_devices=NCORE)

    def din(name, shape, dt=fp16):
        return nc.dram_tensor(name, shape, dt, kind="ExternalInput").ap()

    x0 = din("x0", [SEG, ROWE])
    x1 = din("x1", [SEG, ROWE])
    ix = din("ix", [P, 8 * totck], mybir.dt.int16)
    dm = din("dm", [P, totck])
    wv = din("wv", [P, totck])
    wp = din("wp", [P, 9 * P])
    w2 = din("w2", [P, 3 * P])
    att1w = din("att1w", [P, P])
    a2w = din("a2w", [P, 12 * T])
    outw = din("outw", [P, OUT])
    bz = din("bz", [P, 1], f32)
    br = din("br", [P, 1], f32)
    bh = din("bh", [P, 1], f32)
    ab1 = din("ab1", [P, 1], f32)
    outb = din("outb", [OUT, 1], f32)
    iota = din("iota", [P, P])
    ident = din("ident", [P, P])
    ones16 = din("ones16", [16, P])
    sel12 = din("sel12", [T, T * P])
    out_d = nc.dram_tensor("out", [OUT, YW], f32, kind="ExternalOutput").ap()

    ckmax = int(CK.max())

    with tile.TileContext(nc) as tc:
        with tc.tile_pool(name="const", bufs=1) as cp, \
             tc.tile_pool(name="work", bufs=1) as wkp, \
             tc.tile_pool(name="ps", bufs=1, space="PSUM") as ps:

            def load(ap_in, shape, dt=fp16, name="c"):
                t = cp.tile(shape, dt, tag=name, name=name)
                nc.sync.dma_start(out=t[:], in_=ap_in[:])
                return t

            ix_sb = load(ix, [P, 8 * totck], mybir.dt.int16, "ix")
            dm_sb = load(dm, [P, totck], fp16, "dm")
            wv_sb = load(wv, [P, totck], fp16, "wv")
            wp_sb = load(wp, [P, 9 * P], fp16, "wp")
            w2_sb = load(w2, [P, 3 * P], fp16, "w2")
            a1_sb = load(att1w, [P, P], fp16, "a1")
            a2w_sb = load(a2w, [P, 12 * T], fp16, "a2w")
            ow_sb = load(outw, [P, OUT], fp16, "ow")
            bz_sb = load(bz, [P, 1], f32, "bz")
            br_sb = load(br, [P, 1], f32, "br")
            bh_sb = load(bh, [P, 1], f32, "bh")
            ab1_sb = load(ab1, [P, 1], f32, "ab1")
            ob_sb = load(outb, [OUT, 1], f32, "ob")
            io_sb = load(iota, [P, P], fp16, "io")
            id_sb = load(ident, [P, P], fp16, "id")
            on_sb = load(ones16, [16, P], fp16, "on")
            sel_sb = load(sel12, [T, T * P], fp16, "sel")



            # persistent Y tiles: per node-tile, trios t0-2/t3-5 at rows 0/64
            y0s = [cp.tile([P, NT], fp16, tag=f"y0_{i}", name=f"y0_{i}")
                   for i in range(NNT)]
            y1s = [cp.tile([P, NT], fp16, tag=f"y1_{i}", name=f"y1_{i}")
                   for i in range(NNT)]
            nc.vector.memset(y0s[12][:], 0)
            nc.vector.memset(y1s[12][:], 0)

            # ---------------- phase B: aggregation ----------------
            def emit_block(b):
                ck = int(CK[b])
                klo = int(KLO[b])
                khi = int(KHI[b])
                c0 = int(ck0[b])
                xg = wkp.tile([P, ckmax * ROWE], fp16, tag="xg", bufs=XGB,
                              name=f"xg{b}")

                # prepare-only gathers: GpSimd writes descriptors, the
                # trigger fires them on the DMA engines (idx groups are
                # 128-padded so every chunk is fully overwritten)
                def gather(src, dst_ck, icol0, nidx):
                    done = 0
                    while done < nidx:
                        n = min(1024, nidx - done)
                        o = dst_ck + done // P
                        nck = n // P
                        nc.gpsimd.dma_gather(
                            out_ap=xg[:, o * ROWE:(o + nck) * ROWE]
                                .rearrange("p (c d) -> p c d", c=nck),
                            in_ap=src[:],
                            idxs_ap=ix_sb[:, 8 * (icol0 + done // P):
                                          8 * (icol0 + done // P) + n // 16],
                            num_idxs=n, num_idxs_reg=n,
                            elem_size=ROWE)
                        done += n

                gather(x0, 0, c0, klo * P)
                gather(x1, klo, c0 + klo, khi * P)

                # S_w[p, c, d] = (dm[p,c] == d) * wv[p,c]
                sw = wkp.tile([P, ck * P], fp16, tag="sw", bufs=3,
                              name=f"sw{b}", padded_shape=[P, ckmax * P])
                s3 = sw[:].rearrange("p (c d) -> p c d", c=ck)
                iota_bc = bass.AP(io_sb.tensor, 0, [[P, P], [0, ck], [1, P]])
                nc.vector.tensor_tensor(
                    out=s3, in0=iota_bc,
                    in1=dm_sb[:, c0:c0 + ck].to_broadcast([P, ck, P]),
                    op=OP.is_equal)
                nc.vector.tensor_tensor(
                    out=s3, in0=s3,
                    in1=wv_sb[:, c0:c0 + ck].to_broadcast([P, ck, P]),
                    op=OP.mult)

                # Y_block[d, f] = sum_c S_c^T @ Xg_c   (node-major)
                yps = ps.tile([P, FD], f32, tag="pB", name="yps")
                for c in range(ck):
                    nc.tensor.matmul(
                        out=yps[:], lhsT=sw[:, c * P:(c + 1) * P],
                        rhs=xg[:, c * ROWE:c * ROWE + FD],
                        start=(c == 0), stop=(c == ck - 1))
                yb = wkp.tile([P, FD], fp16, tag="yb", bufs=2, name=f"yb{b}")
                nc.scalar.copy(out=yb[:], in_=yps[:])

                # transpose 48-col groups to [48, 128] at bases 0/64
                nt_i, csl = b // 4, slice((b % 4) * P, (b % 4 + 1) * P)
                for half, yt in ((0, y0s[nt_i]), (1, y1s[nt_i])):
                    tp = ps.tile([P, P], fp16, tag="pB", name="tt")
                    for g in (2 * half, 2 * half + 1):
                        nc.tensor.transpose(out=tp[64 * (g % 2):64 * (g % 2) + 48, :],
                                            in_=yb[:, g * 48:(g + 1) * 48],
                                            identity=id_sb[:])
                    nc.scalar.copy(out=yt[0:48, csl], in_=tp[0:48, :])
                    nc.scalar.copy(out=yt[64:112, csl], in_=tp[64:112, :])

            # ---------------- phase C: GRU + attention ----------------
            def tile_state(i):
                w = 128 if i == NNT - 1 else NT    # last tile: 106 real nodes
                scp = ps.tile([T, NT], f32, tag="scp", bufs=2,
                              name=f"scp{i}")
                return dict(i=i, w=w, hts=[], scp=scp)

            def emit_step(st, t):
                    i, hts, scp, w = st["i"], st["hts"], st["scp"], st["w"]
                    v = t % 3
                    base = 64 * ((t % 6) // 3)
                    yt = (y0s[i] if t < 6 else y1s[i])[base:base + 48, :w]
                    hprev = hts[-1] if t else None

                    def ypart(gi, pre, stop):
                        nc.tensor.matmul(
                            out=pre[:, :w],
                            lhsT=wp_sb[base:base + 48,
                                       (gi * 3 + v) * P:(gi * 3 + v + 1) * P],
                            rhs=yt, start=True, stop=stop)

                    # critical path first: r branch feeds hpre
                    # h~ = tanh(Wh1.y + (Wh2/2).((u_r+1)*h) + bh)
                    #    = tanh(Wh1.y + Wh2.(r*h) + bh)
                    hpre = ps.tile([P, NT], f32, tag="big", bufs=4, name="hpre")
                    if t:
                        rpre = ps.tile([P, NT], f32, tag="big", bufs=4,
                                       name="rpre")
                        ypart(1, rpre, False)
                        nc.tensor.matmul(out=rpre[:, :w], lhsT=w2_sb[:, P:2 * P],
                                         rhs=hprev[:, :w], start=False, stop=True)
                        ur = wkp.tile([P, NT], fp16, tag="ur", bufs=2,
                                      name="ur")
                        nc.scalar.activation(out=ur[:, :w], in_=rpre[:, :w],
                                             func=AF.Tanh, bias=br_sb[:],
                                             scale=0.5)
                        urh = wkp.tile([P, NT], fp16, tag="urh", bufs=2,
                                       name="urh")
                        nc.vector.scalar_tensor_tensor(
                            out=urh[:, :w], in0=ur[:, :w], scalar=1.0,
                            in1=hprev[:, :w], op0=OP.add, op1=OP.mult)
                    ypart(2, hpre, t == 0)
                    if t:
                        nc.tensor.matmul(out=hpre[:, :w], lhsT=w2_sb[:, 2 * P:3 * P],
                                         rhs=urh[:, :w], start=False, stop=True)

                    # off-path: u_z = tanh((zpre + bz)/2);  z = (1 + u_z)/2
                    zpre = ps.tile([P, NT], f32, tag="big", bufs=4, name="zpre")
                    ypart(0, zpre, t == 0)
                    if t:
                        nc.tensor.matmul(out=zpre[:, :w], lhsT=w2_sb[:, 0:P],
                                         rhs=hprev[:, :w], start=False, stop=True)
                    uz = wkp.tile([P, NT], fp16, tag="uz", bufs=2, name="uz")
                    nc.scalar.activation(out=uz[:, :w], in_=zpre[:, :w],
                                         func=AF.Tanh, bias=bz_sb[:], scale=0.5)
                    ht_ = wkp.tile([P, NT], fp16, tag="ht", bufs=2, name="ht")
                    nc.scalar.activation(out=ht_[:, :w], in_=hpre[:, :w],
                                         func=AF.Tanh, bias=bh_sb[:])

                    # h' = h~ + z*(h - h~)  with  z*(h-h~) = 0.5*(u_z+1)*d
                    d = wkp.tile([P, NT], fp16, tag="d", bufs=2, name="d")
                    if t == 0:
                        nc.vector.tensor_scalar(out=d[:, :w], in0=ht_[:, :w],
                                                scalar1=-1.0, scalar2=None,
                                                op0=OP.mult)
                    else:
                        nc.vector.tensor_tensor(out=d[:, :w], in0=hprev[:, :w],
                                                in1=ht_[:, :w], op=OP.subtract)
                    zd = wkp.tile([P, NT], fp16, tag="zd", bufs=2, name="zd")
                    nc.vector.scalar_tensor_tensor(
                        out=zd[:, :w], in0=uz[:, :w], scalar=1.0,
                        in1=d[:, :w], op0=OP.add, op1=OP.mult)
                    h = wkp.tile([P, NT], fp16, tag=f"h{t}", bufs=2,
                                 name=f"h{t}_{i}")
                    nc.vector.scalar_tensor_tensor(
                        out=h[:, :w], in0=zd[:, :w], scalar=0.5,
                        in1=ht_[:, :w], op0=OP.mult, op1=OP.add)
                    hts.append(h)

                    # attention score: accumulate row t of the [12, NT]
                    # score tile via the masked a2 lhsT (column t = att2)
                    apre = ps.tile([P, NT], f32, tag="big", bufs=4, name="apre")
                    nc.tensor.matmul(out=apre[:, :w], lhsT=a1_sb[:], rhs=h[:, :w],
                                     start=True, stop=True)
                    relu1 = wkp.tile([P, NT], fp16, tag="relu1", bufs=2,
                                     name="relu1")
                    nc.scalar.activation(out=relu1[:, :w], in_=apre[:, :w],
                                         func=AF.Relu, bias=ab1_sb[:])
                    nc.tensor.matmul(out=scp[:, :w],
                                     lhsT=a2w_sb[:, t * T:(t + 1) * T],
                                     rhs=relu1[:, :w],
                                     start=(t == 0), stop=(t == T - 1))

            def emit_tail(st):
                i, hts, scp, w = st["i"], st["hts"], st["scp"], st["w"]
                # batched exp over all 12 score rows (att2_b dropped: a
                # constant bias cancels in softmax)
                ets = wkp.tile([T, NT], fp16, tag="ets", bufs=2,
                               name=f"ets{i}")
                nc.scalar.activation(out=ets[:, :w], in_=scp[:, :w],
                                     func=AF.Exp)
                # softmax denominator + reciprocal, all off GpSimd so its
                # stream stays pure gather desc-gen
                rec = wkp.tile([1, NT], f32, tag="rec", bufs=2, name="rec")
                dps = ps.tile([1, NT], f32, tag="dps", bufs=1, name="dps")
                nc.tensor.matmul(out=dps[:, :w], lhsT=on_sb[0:T, 0:1],
                                 rhs=ets[:, :w], start=True, stop=True)
                nc.vector.reciprocal_approx_fast(out=rec[:, :w],
                                                 in_=dps[:, :w])
                rec16 = wkp.tile([1, NT], fp16, tag="rec16", bufs=2,
                                 name="rec16")
                nc.scalar.copy(out=rec16[:, :w], in_=rec[:, :w])

                # softmax-weighted h sum: broadcast exp row t across all 128
                # partitions with a select-matmul (lhsT block t = ones row t,
                # reading ets at partition 0), then MAC on DVE
                acc = wkp.tile([P, NT], fp16, tag="acc", bufs=2, name=f"acc{i}")
                for t in range(T):
                    ebc = ps.tile([P, NT], f32, tag="big", bufs=4, name="ebc")
                    nc.tensor.matmul(out=ebc[:, :w],
                                     lhsT=sel_sb[:, t * P:(t + 1) * P],
                                     rhs=ets[:, :w], start=True, stop=True)
                    if t == 0:
                        nc.vector.tensor_tensor(out=acc[:, :w], in0=hts[0][:, :w],
                                                in1=ebc[:, :w], op=OP.mult)
                    else:
                        tmp = wkp.tile([P, NT], fp16, tag="tmp", bufs=2,
                                       name="tmp")
                        nc.vector.tensor_tensor(out=tmp[:, :w], in0=hts[t][:, :w],
                                                in1=ebc[:, :w], op=OP.mult)
                        nc.vector.tensor_tensor(out=acc[:, :w], in0=acc[:, :w],
                                                in1=tmp[:, :w], op=OP.add)

                # 1/den commutes through the output projection (per-node
                # scalar): out = (ow^T @ acc) * (1/den) + ob
                ops = ps.tile([OUT, NT], f32, tag="big", bufs=4, name="ops")
                nc.tensor.matmul(out=ops[:, :w], lhsT=ow_sb[:],
                                 rhs=acc[:, :w], start=True, stop=True)
                rbc = ps.tile([OUT, NT], f32, tag="big", bufs=4, name="rbc")
                nc.tensor.matmul(out=rbc[:, :w], lhsT=on_sb[0:1, 0:OUT],
                                 rhs=rec16[:, :w], start=True, stop=True)
                rbs = wkp.tile([OUT, NT], f32, tag="rbs", bufs=2, name="rbs")
                nc.scalar.copy(out=rbs[:, :w], in_=rbc[:, :w])
                osb = wkp.tile([OUT, NT], f32, tag="osb", bufs=2, name="osb")
                nc.vector.tensor_tensor(out=osb[:, :w], in0=ops[:, :w],
                                        in1=rbs[:, :w], op=OP.mult)
                nc.vector.tensor_scalar(out=osb[:, :w], in0=osb[:, :w],
                                        scalar1=ob_sb[:], scalar2=None,
                                        op0=OP.add)
                nc.sync.dma_start(out=out_d[:, i * NT:i * NT + w],
                                  in_=osb[:, :w])

            # interleave: emit each pair of node-tiles right after their
            # source blocks, alternating the two tiles' GRU steps so every
            # engine's static order has two independent dependency chains
            # in flight (hides per-step latency); phase-C compute overlaps
            # the phase-B gathers
            # software-pipelined: emit group g+1's gather/aggregation blocks
            # BEFORE group g's GRU so desc-gen and DMA for the next group
            # overlap this group's compute
            groups = [[i for i in (g0, g0 + 1) if i < NNT]
                      for g0 in range(0, NNT, 2)]

            def emit_group_blocks(tiles):
                for i in tiles:
                    for b in range(4 * i, min(4 * i + 4, NBLK)):
                        emit_block(b)

            emit_group_blocks(groups[0])
            for gi, tiles in enumerate(groups):
                if gi + 1 < len(groups):
                    emit_group_blocks(groups[gi + 1])
                sts = [tile_state(i) for i in tiles]
                for t in range(T):
                    for st in sts:
                        emit_step(st, t)
                for st in sts:
                    emit_tail(st)

    nc.finalize()
    return nc


def kernel(**inputs):
    from concourse import bass_utils

    x = np.asarray(inputs["x"], np.float32)
    dm_arr, wv_arr, ix_arr, x0, x1, KLO, KHI, CK, ck0 = _host_prep(
        x, np.asarray(inputs["edge_index"]), np.asarray(inputs["edge_weight"]))
    wts = _fold_weights({k: np.asarray(v) for k, v in inputs.items()})
    totck = int(ck0[-1])

    nc = _build_graph(totck, KLO, KHI, CK, ck0)

    iota = np.broadcast_to(np.arange(P, dtype=np.float16), (P, P)).copy()
    ident = np.eye(P, dtype=np.float16)
    ones16 = np.ones((16, P), np.float16)
    sel12 = np.zeros((T, T * P), np.float16)
    for t in range(T):
        sel12[t, t * P:(t + 1) * P] = 1.0
    shared = dict(x0=x0, x1=x1, iota=iota, ident=ident, ones16=ones16,
                  sel12=sel12, **wts)
    in_maps = [dict(ix=ix_arr[c], dm=dm_arr[c], wv=wv_arr[c], **shared)
               for c in range(NCORE)]

    res = bass_utils.run_bass_kernel_spmd(
        nc, in_maps, core_ids=list(range(NCORE)))
    kernel._last_results = res
    out = np.concatenate(
        [np.asarray(res.results[c]["out"]).T[:PERCORE] for c in range(NCORE)])
    return np.ascontiguousarray(out, dtype=np.float32)


# revision 33
# speedup vs baseline: 1.0231x; 1.0231x over previous
"""A3TGCN (attention temporal GCN) on 8 Trainium2 NeuronCores.

Strategy
--------
The reference computes, per timestep t, three GCN convolutions of the form
segment_sum(norm * (x_t @ W)[src]) which commute with the dense projection:
  A_hat @ (x_t @ W) == (A_hat @ x_t) @ W.
All three convs at every timestep therefore share ONE sparse aggregation
Y = A_hat @ X with X = concat_t x_t  (50k x 192), after which the whole
GRU recurrence + attention is purely node-local dense compute.

Sharding: nodes are partitioned across the 8 cores on the dst axis
(6250 real + 22 pad = 6272 per core). Inputs (X, weights) are replicated,
so no halo exchange is needed at runtime; each core gathers the source
rows for its in-edges directly from HBM via dma_gather.

Host-side one-time preprocessing (graph/weight setup, as in any deployed
GNN): GCN normalization w' = dinv[src]*w*dinv[dst] (the reference marks
this "computed once"), GRU weight folding
  Wz1 = conv_z_w @ lin_z_w[:H],  bz = conv_z_b @ lin_z_w[:H] + lin_z_b
(ditto r/h), and edge bucketing/padding into the fixed device layout.
All O(E*F*T) work — aggregation, GRU, attention, output projection —
runs on device.

Device pipeline per core:
  phase B, per 128-dst block: dma_gather source rows (fp16, 512B rows,
    int16 indices over two 25088-row segments of X, idx groups padded to
    128 so every SBUF chunk is fully overwritten — no memsets, no NaNs).
    Gathers run in PREPARE_ONLY mode: GpSimd only generates descriptors
    and trigger_dma fires them on the 16 DMA engines, so desc-gen for
    block b+1 overlaps the transfer for block b instead of serializing.
    The scatter matrix S_w[e, dmod] = w'_e builds on DVE (broadcast
    is_equal * w); Y_block = S_w^T @ Xg accumulates in PSUM over edge
    chunks, then 48-feature groups transpose so timestep trios sit at
    partition offsets 0/64.
  phase C, per 512-node tile: 12 GRU steps (fp16 matmuls, f32 PSUM).
    sigmoid(x) = (1+tanh(x/2))/2 keeps every ACT function (tanh/relu/
    copy/exp) in ONE activation table. scalar_tensor_tensor fuses the
    GRU update: urh2=(u_r+1)*h gives Wh2*(r*h) in ONE matmul on the
    halved Wh2 slot, and h' = h~ + 0.5*(u_z+1)*(h-h~) needs no separate
    z tile. Attention scores accumulate into a single [12,NT] PSUM tile
    via per-step masked-lhsT matmuls (lhsT column t = att2), so exp runs
    ONCE per tile over [12,NT]; the softmax denominator reduces on
    GpSimd (partition_all_reduce) and the exp rows broadcast back on
    GpSimd (partition_broadcast) for the DVE weighted-h MAC. The
    constant att2 bias cancels in softmax and is dropped; 1/den uses
    the fast DVE reciprocal (|err| ~1e-5 rel, gate is 2e-2).
"""

import numpy as np

N, F, T, H, OUT = 50000, 16, 12, 128, 16
NCORE = 8
PERCORE = N // NCORE            # 6250 real nodes per core
BLK = 128
NBLK = 49                       # 49 blocks of 128 = 6272 padded nodes/core
SEG = 25088                     # X segment rows (int16-indexable)
ROWE = 256                      # fp16 elements per X row (192 data + pad)
FD = F * T                      # 192
P = 128
NT = 512                        # node-tile width in phase C
NNT = 13                        # node tiles per core (13*512 = 6656 >= 6272)
YW = NNT * NT                   # padded out width
USE_GPB = True                  # softmax denominator reduce on GpSimd
XGB = 8                         # xg gather buffers: full group in flight


def _host_prep(x, edge_index, edge_weight):
    """Graph normalization + device data layout. Returns per-core arrays."""
    src = np.concatenate([edge_index[0].astype(np.int64), np.arange(N)])
    dst = np.concatenate([edge_index[1].astype(np.int64), np.arange(N)])
    w = np.concatenate([edge_weight.astype(np.float32), np.ones(N, np.float32)])

    deg = np.bincount(dst, weights=w, minlength=N).astype(np.float32)
    dinv = np.where(deg > 0, 1.0 / np.sqrt(deg), 0.0).astype(np.float32)
    wn = (dinv[src] * w * dinv[dst]).astype(np.float32)

    core = dst // PERCORE
    rloc = dst % PERCORE
    blk = rloc // BLK
    dmod = (rloc % BLK).astype(np.float16)
    seg = (src >= SEG).astype(np.int64)
    idx16 = (src - SEG * seg).astype(np.int16)

    key = (core * NBLK + blk) * 2 + seg
    order = np.argsort(key, kind="stable")
    key_s = key[order]
    cnt = np.bincount(key, minlength=NCORE * NBLK * 2).reshape(NCORE, NBLK, 2)
    # chunks per (block, segment): max over cores so the SPMD graph is shared
    maxc = cnt.max(0)                                          # [NBLK, 2]
    KLO = np.maximum(1, -(-maxc[:, 0] // P))
    KHI = np.maximum(1, -(-maxc[:, 1] // P))
    CK = KLO + KHI
    ck0 = np.zeros(NBLK + 1, np.int64)
    ck0[1:] = np.cumsum(CK)

    # position of each edge within its (core, blk, seg) group
    gstart = np.zeros(NCORE * NBLK * 2 + 1, np.int64)
    gstart[1:] = np.cumsum(cnt.ravel())
    j = np.arange(len(order)) - gstart[key_s]

    core_s = core[order]
    blk_s = blk[order]
    seg_s = seg[order]
    idx_s = idx16[order]
    dmod_s = dmod[order]
    wn_s = wn[order].astype(np.float16)

    totck = int(ck0[-1])
    # global chunk column and lane for the (dmod, w) slot layout
    gchunk = ck0[blk_s] + KLO[blk_s] * seg_s + j // P
    lane = j % P
    # wrapped idx layout: within a gather group, index k sits at
    # [k%16, k//16]; gather for (blk b, seg s) owns cols [8*(ck0+KLO*s)...)
    icol = 8 * (ck0[blk_s] + KLO[blk_s] * seg_s) + j // 16
    irow = j % 16

    dm_arr, wv_arr, ix_arr = [], [], []
    for c in range(NCORE):
        m = core_s == c
        dmc = np.zeros((P, totck), np.float16)
        wvc = np.zeros((P, totck), np.float16)
        dmc[lane[m], gchunk[m]] = dmod_s[m]
        wvc[lane[m], gchunk[m]] = wn_s[m]
        # idx arrays default to 0: every lane of every chunk gathers a
        # real row (row 0 for pads), so chunks are always fully written
        ixc = np.zeros((16, totck * 8), np.int16)
        ixc[irow[m], icol[m]] = idx_s[m]

        dm_arr.append(dmc)
        wv_arr.append(wvc)
        ix_arr.append(np.tile(ixc, (8, 1)))

    # X replica: row n = x[n] laid out t-major (col = t*F + f), fp16, padded
    xt = np.zeros((2 * SEG, ROWE), np.float16)
    xt[:N, :FD] = np.moveaxis(x, 2, 1).reshape(N, FD)
    return dm_arr, wv_arr, ix_arr, xt[:SEG], xt[SEG:], KLO, KHI, CK, ck0


def _fold_weights(inp):
    """Fold conv into lin weights (the concat trick) and build the padded
    timestep-trio lhsT variants used by phase C."""
    f32 = np.float32
    wz1 = inp["conv_z_w"].astype(f32) @ inp["lin_z_w"].astype(f32)[:H]
    wr1 = inp["conv_r_w"].astype(f32) @ inp["lin_r_w"].astype(f32)[:H]
    wh1 = inp["conv_h_w"].astype(f32) @ inp["lin_h_w"].astype(f32)[:H]
    bz = inp["conv_z_b"].astype(f32) @ inp["lin_z_w"].astype(f32)[:H] + inp["lin_z_b"].astype(f32)
    br = inp["conv_r_b"].astype(f32) @ inp["lin_r_w"].astype(f32)[:H] + inp["lin_r_b"].astype(f32)
    bh = inp["conv_h_b"].astype(f32) @ inp["lin_h_w"].astype(f32)[:H] + inp["lin_h_b"].astype(f32)

    # wp[:, (gate*3+v)*128 : +128]: rows [16v:16v+16] and [64+16v:+16] = W
    wp = np.zeros((P, 9 * P), np.float16)
    for g, w1 in enumerate((wz1, wr1, wh1)):
        for v in range(3):
            col = (g * 3 + v) * P
            wp[16 * v:16 * v + 16, col:col + P] = w1.astype(np.float16)
            wp[64 + 16 * v:64 + 16 * v + 16, col:col + P] = w1.astype(np.float16)
    # gate-h slot halved: Wh2*(r*h) is computed as (Wh2/2) @ ((u_r+1)*h)
    w2 = np.concatenate([inp["lin_z_w"][H:], inp["lin_r_w"][H:],
                         inp["lin_h_w"][H:] * 0.5], axis=1).astype(np.float16)
    # per-step score lhsT: block t has att2 in column t, zeros elsewhere,
    # so 12 accumulating matmuls build the [12, NT] score tile in place
    a2w = np.zeros((P, 12 * T), np.float16)
    for t in range(T):
        a2w[:, t * T + t] = inp["att2_w"].astype(np.float16).reshape(H)
    return dict(
        wp=wp, w2=w2,
        att1w=inp["att1_w"].astype(np.float16),
        a2w=a2w,
        outw=inp["out_w"].astype(np.float16),
        bz=(bz / 2).reshape(H, 1).astype(f32),
        br=(br / 2).reshape(H, 1).astype(f32),
        bh=bh.reshape(H, 1).astype(f32),
        ab1=inp["att1_b"].astype(f32).reshape(H, 1),
        outb=inp["out_b"].astype(f32).reshape(OUT, 1),
    )


def _build_graph(totck, KLO, KHI, CK, ck0):
    from concourse import bass, bacc, mybir
    import concourse.tile as tile
    import concourse.bass_isa as bass_isa

    fp16 = mybir.dt.float16
    f32 = mybir.dt.float32
    AF = mybir.ActivationFunctionType
    OP = mybir.AluOpType

    nc = bacc.Bacc("TRN2", target_bir_lowering=False, debug=False,
                   num_devices=NCORE)

    def din(name, shape, dt=fp16):
        return nc.dram_tensor(name, shape, dt, kind="ExternalInput").ap()

    x0 = din("x0", [SEG, ROWE])
    x1 = din("x1", [SEG, ROWE])
    ix = din("ix", [P, 8 * totck], mybir.dt.int16)
    dm = din("dm", [P, totck])
    wv = din("wv", [P, totck])
    wp = din("wp", [P, 9 * P])
    w2 = din("w2", [P, 3 * P])
    att1w = din("att1w", [P, P])
    a2w = din("a2w", [P, 12 * T])
    outw = din("outw", [P, OUT])
    bz = din("bz", [P, 1], f32)
    br = din("br", [P, 1], f32)
    bh = din("bh", [P, 1], f32)
    ab1 = din("ab1", [P, 1], f32)
    outb = din("outb", [OUT, 1], f32)
    iota = din("iota", [P, P])
    ident = din("ident", [P, P])
    ones16 = din("ones16", [16, P])
    sel12 = din("sel12", [T, T * P])
    out_d = nc.dram_tensor("out", [OUT, YW], f32, kind="ExternalOutput").ap()

    ckmax = int(CK.max())

    with tile.TileContext(nc) as tc:
        with tc.tile_pool(name="const", bufs=1) as cp, \
             tc.tile_pool(name="work", bufs=1) as wkp, \
             tc.tile_pool(name="ps", bufs=1, space="PSUM") as ps:

            def load(ap_in, shape, dt=fp16, name="c"):
                t = cp.tile(shape, dt, tag=name, name=name)
                nc.sync.dma_start(out=t[:], in_=ap_in[:])
                return t

            ix_sb = load(ix, [P, 8 * totck], mybir.dt.int16, "ix")
            dm_sb = load(dm, [P, totck], fp16, "dm")
            wv_sb = load(wv, [P, totck], fp16, "wv")
            wp_sb = load(wp, [P, 9 * P], fp16, "wp")
            w2_sb = load(w2, [P, 3 * P], fp16, "w2")
            a1_sb = load(att1w, [P, P], fp16, "a1")
            a2w_sb = load(a2w, [P, 12 * T], fp16, "a2w")
            ow_sb = load(outw, [P, OUT], fp16, "ow")
            bz_sb = load(bz, [P, 1], f32, "bz")
            br_sb = load(br, [P, 1], f32, "br")
            bh_sb = load(bh, [P, 1], f32, "bh")
            ab1_sb = load(ab1, [P, 1], f32, "ab1")
            ob_sb = load(outb, [OUT, 1], f32, "ob")
            io_sb = load(iota, [P, P], fp16, "io")
            id_sb = load(ident, [P, P], fp16, "id")
            on_sb = load(ones16, [16, P], fp16, "on")
            sel_sb = load(sel12, [T, T * P], fp16, "sel")



            # persistent Y tiles: per node-tile, trios t0-2/t3-5 at rows 0/64
            y0s = [cp.tile([P, NT], fp16, tag=f"y0_{i}", name=f"y0_{i}")
                   for i in range(NNT)]
            y1s = [cp.tile([P, NT], fp16, tag=f"y1_{i}", name=f"y1_{i}")
                   for i in range(NNT)]
            nc.vector.memset(y0s[12][:], 0)
            nc.vector.memset(y1s[12][:], 0)

            # ---------------- phase B: aggregation ----------------
            def emit_block(b):
                ck = int(CK[b])
                klo = int(KLO[b])
                khi = int(KHI[b])
                c0 = int(ck0[b])
                xg = wkp.tile([P, ckmax * ROWE], fp16, tag="xg", bufs=XGB,
                              name=f"xg{b}")

                # prepare-only gathers: GpSimd writes descriptors, the
                # trigger fires them on the DMA engines (idx groups are
                # 128-padded so every chunk is fully overwritten)
                def gather(src, dst_ck, icol0, nidx):
                    done = 0
                    while done < nidx:
                        n = min(1024, nidx - done)
                        o = dst_ck + done // P
                        nck = n // P
                        nc.gpsimd.dma_gather(
                            out_ap=xg[:, o * ROWE:(o + nck) * ROWE]
                                .rearrange("p (c d) -> p c d", c=nck),
                            in_ap=src[:],
                            idxs_ap=ix_sb[:, 8 * (icol0 + done // P):
                                          8 * (icol0 + done // P) + n // 16],
                            num_idxs=n, num_idxs_reg=n,
                            elem_size=ROWE)
                        done += n

                gather(x0, 0, c0, klo * P)
                gather(x1, klo, c0 + klo, khi * P)

                # S_w[p, c, d] = (dm[p,c] == d) * wv[p,c]
                sw = wkp.tile([P, ck * P], fp16, tag="sw", bufs=2,
                              name=f"sw{b}", padded_shape=[P, ckmax * P])
                s3 = sw[:].rearrange("p (c d) -> p c d", c=ck)
                iota_bc = bass.AP(io_sb.tensor, 0, [[P, P], [0, ck], [1, P]])
                nc.vector.tensor_tensor(
                    out=s3, in0=iota_bc,
                    in1=dm_sb[:, c0:c0 + ck].to_broadcast([P, ck, P]),
                    op=OP.is_equal)
                nc.vector.tensor_tensor(
                    out=s3, in0=s3,
                    in1=wv_sb[:, c0:c0 + ck].to_broadcast([P, ck, P]),
                    op=OP.mult)

                # Y_block[d, f] = sum_c S_c^T @ Xg_c   (node-major)
                yps = ps.tile([P, FD], f32, tag="pB", name="yps")
                for c in range(ck):
                    nc.tensor.matmul(
                        out=yps[:], lhsT=sw[:, c * P:(c + 1) * P],
                        rhs=xg[:, c * ROWE:c * ROWE + FD],
                        start=(c == 0), stop=(c == ck - 1))
                yb = wkp.tile([P, FD], fp16, tag="yb", bufs=2, name=f"yb{b}")
                nc.scalar.copy(out=yb[:], in_=yps[:])

                # transpose 48-col groups to [48, 128] at bases 0/64
                nt_i, csl = b // 4, slice((b % 4) * P, (b % 4 + 1) * P)
                for half, yt in ((0, y0s[nt_i]), (1, y1s[nt_i])):
                    tp = ps.tile([P, P], fp16, tag="pB", name="tt")
                    for g in (2 * half, 2 * half + 1):
                        nc.tensor.transpose(out=tp[64 * (g % 2):64 * (g % 2) + 48, :],
                                            in_=yb[:, g * 48:(g + 1) * 48],
                                            identity=id_sb[:])
                    nc.scalar.copy(out=yt[0:48, csl], in_=tp[0:48, :])
                    nc.scalar.copy(out=yt[64:112, csl], in_=tp[64:112, :])

            # ---------------- phase C: GRU + attention ----------------
            def tile_state(i):
                w = 128 if i == NNT - 1 else NT    # last tile: 106 real nodes
                scp = ps.tile([T, NT], f32, tag="scp", bufs=2,
                              name=f"scp{i}")
                return dict(i=i, w=w, hts=[], scp=scp)

            def emit_step(st, t):
                    i, hts, scp, w = st["i"], st["hts"], st["scp"], st["w"]
                    v = t % 3
                    base = 64 * ((t % 6) // 3)
                    yt = (y0s[i] if t < 6 else y1s[i])[base:base + 48, :w]
                    hprev = hts[-1] if t else None

                    def ypart(gi, pre, stop):
                        nc.tensor.matmul(
                            out=pre[:, :w],
                            lhsT=wp_sb[base:base + 48,
                                       (gi * 3 + v) * P:(gi * 3 + v + 1) * P],
                            rhs=yt, start=True, stop=stop)

                    # critical path first: r branch feeds hpre
                    # h~ = tanh(Wh1.y + (Wh2/2).((u_r+1)*h) + bh)
                    #    = tanh(Wh1.y + Wh2.(r*h) + bh)
                    hpre = ps.tile([P, NT], f32, tag="big", bufs=4, name="hpre")
                    if t:
                        rpre = ps.tile([P, NT], f32, tag="big", bufs=4,
                                       name="rpre")
                        ypart(1, rpre, False)
                        nc.tensor.matmul(out=rpre[:, :w], lhsT=w2_sb[:, P:2 * P],
                                         rhs=hprev[:, :w], start=False, stop=True)
                        ur = wkp.tile([P, NT], fp16, tag="ur", bufs=2,
                                      name="ur")
                        nc.scalar.activation(out=ur[:, :w], in_=rpre[:, :w],
                                             func=AF.Tanh, bias=br_sb[:],
                                             scale=0.5)
                        urh = wkp.tile([P, NT], fp16, tag="urh", bufs=2,
                                       name="urh")
                        nc.vector.scalar_tensor_tensor(
                            out=urh[:, :w], in0=ur[:, :w], scalar=1.0,
                            in1=hprev[:, :w], op0=OP.add, op1=OP.mult)
                    ypart(2, hpre, t == 0)
                    if t:
                        nc.tensor.matmul(out=hpre[:, :w], lhsT=w2_sb[:, 2 * P:3 * P],
                                         rhs=urh[:, :w], start=False, stop=True)

                    # off-path: u_z = tanh((zpre + bz)/2);  z = (1 + u_z)/2
                    zpre = ps.tile([P, NT], f32, tag="big", bufs=4, name="zpre")
                    ypart(0, zpre, t == 0)
                    if t:
                        nc.tensor.matmul(out=zpre[:, :w], lhsT=w2_sb[:, 0:P],
                                         rhs=hprev[:, :w], start=False, stop=True)
                    uz = wkp.tile([P, NT], fp16, tag="uz", bufs=2, name="uz")
                    nc.scalar.activation(out=uz[:, :w], in_=zpre[:, :w],
                                         func=AF.Tanh, bias=bz_sb[:], scale=0.5)
                    ht_ = wkp.tile([P, NT], fp16, tag="ht", bufs=2, name="ht")
                    nc.scalar.activation(out=ht_[:, :w], in_=hpre[:, :w],
                                         func=AF.Tanh, bias=bh_sb[:])

                    # h' = h~ + z*(h - h~)  with  z*(h-h~) = 0.5*(u_z+1)*d
                    d = wkp.tile([P, NT], fp16, tag="d", bufs=2, name="d")
                    if t == 0:
                        nc.vector.tensor_scalar(out=d[:, :w], in0=ht_[:, :w],
                                                scalar1=-1.0, scalar2=None,
                                                op0=OP.mult)
                    else:
                        nc.vector.tensor_tensor(out=d[:, :w], in0=hprev[:, :w],
                                                in1=ht_[:, :w], op=OP.subtract)
                    zd = wkp.tile([P, NT], fp16, tag="zd", bufs=2, name="zd")
                    nc.vector.scalar_tensor_tensor(
                        out=zd[:, :w], in0=uz[:, :w], scalar=1.0,
                        in1=d[:, :w], op0=OP.add, op1=OP.mult)
                    h = wkp.tile([P, NT], fp16, tag=f"h{t}", bufs=2,
                                 name=f"h{t}_{i}")
                    nc.vector.scalar_tensor_tensor(
                        out=h[:, :w], in0=zd[:, :w], scalar=0.5,
                        in1=ht_[:, :w], op0=OP.mult, op1=OP.add)
                    hts.append(h)

                    # attention score: accumulate row t of the [12, NT]
                    # score tile via the masked a2 lhsT (column t = att2)
                    apre = ps.tile([P, NT], f32, tag="big", bufs=4, name="apre")
                    nc.tensor.matmul(out=apre[:, :w], lhsT=a1_sb[:], rhs=h[:, :w],
                                     start=True, stop=True)
                    relu1 = wkp.tile([P, NT], fp16, tag="relu1", bufs=2,
                                     name="relu1")
                    nc.scalar.activation(out=relu1[:, :w], in_=apre[:, :w],
                                         func=AF.Relu, bias=ab1_sb[:])
                    nc.tensor.matmul(out=scp[:, :w],
                                     lhsT=a2w_sb[:, t * T:(t + 1) * T],
                                     rhs=relu1[:, :w],
                                     start=(t == 0), stop=(t == T - 1))

            def emit_tail(st):
                i, hts, scp, w = st["i"], st["hts"], st["scp"], st["w"]
                # batched exp over all 12 score rows (att2_b dropped: a
                # constant bias cancels in softmax)
                ets = wkp.tile([T, NT], fp16, tag="ets", bufs=2,
                               name=f"ets{i}")
                nc.scalar.activation(out=ets[:, :w], in_=scp[:, :w],
                                     func=AF.Exp)
                # softmax denominator + reciprocal, all off GpSimd so its
                # stream stays pure gather desc-gen
                rec = wkp.tile([1, NT], f32, tag="rec", bufs=2, name="rec")
                dps = ps.tile([1, NT], f32, tag="dps", bufs=1, name="dps")
                nc.tensor.matmul(out=dps[:, :w], lhsT=on_sb[0:T, 0:1],
                                 rhs=ets[:, :w], start=True, stop=True)
                nc.vector.reciprocal_approx_fast(out=rec[:, :w],
                                                 in_=dps[:, :w])
                rec16 = wkp.tile([1, NT], fp16, tag="rec16", bufs=2,
                                 name="rec16")
                nc.scalar.copy(out=rec16[:, :w], in_=rec[:, :w])

                # softmax-weighted h sum: broadcast exp row t across all 128
                # partitions with a select-matmul (lhsT block t = ones row t,
                # reading ets at partition 0), then MAC on DVE
                acc = wkp.tile([P, NT], fp16, tag="acc", bufs=2, name=f"acc{i}")
                for t in range(T):
                    ebc = ps.tile([P, NT], f32, tag="big", bufs=4, name="ebc")
                    nc.tensor.matmul(out=ebc[:, :w],
                                     lhsT=sel_sb[:, t * P:(t + 1) * P],
                                     rhs=ets[:, :w], start=True, stop=True)
                    if t == 0:
                        nc.vector.tensor_tensor(out=acc[:, :w], in0=hts[0][:, :w],
                                                in1=ebc[:, :w], op=OP.mult)
                    else:
                        tmp = wkp.tile([P, NT], fp16, tag="tmp", bufs=2,
                                       name="tmp")
                        nc.vector.tensor_tensor(out=tmp[:, :w], in0=hts[t][:, :w],
                                                in1=ebc[:, :w], op=OP.mult)
                        nc.vector.tensor_tensor(out=acc[:, :w], in0=acc[:, :w],
                                                in1=tmp[:, :w], op=OP.add)

                # 1/den commutes through the output projection (per-node
                # scalar): out = (ow^T @ acc) * (1/den) + ob
                ops = ps.tile([OUT, NT], f32, tag="big", bufs=4, name="ops")
                nc.tensor.matmul(out=ops[:, :w], lhsT=ow_sb[:],
                                 rhs=acc[:, :w], start=True, stop=True)
                rbc = ps.tile([OUT, NT], f32, tag="big", bufs=4, name="rbc")
                nc.tensor.matmul(out=rbc[:, :w], lhsT=on_sb[0:1, 0:OUT],
                                 rhs=rec16[:, :w], start=True, stop=True)
                rbs = wkp.tile([OUT, NT], f32, tag="rbs", bufs=2, name="rbs")
                nc.scalar.copy(out=rbs[:, :w], in_=rbc[:, :w])
                osb = wkp.tile([OUT, NT], f32, tag="osb", bufs=2, name="osb")
                nc.vector.tensor_tensor(out=osb[:, :w], in0=ops[:, :w],
                                        in1=rbs[:, :w], op=OP.mult)
                nc.vector.tensor_scalar(out=osb[:, :w], in0=osb[:, :w],
                                        scalar1=ob_sb[:], scalar2=None,
                                        op0=OP.add)
                nc.sync.dma_start(out=out_d[:, i * NT:i * NT + w],
                                  in_=osb[:, :w])

            # interleave: emit each pair of node-tiles right after their
            # source blocks, alternating the two tiles' GRU steps so every
            # engine's static order has two independent dependency chains
            # in flight (hides per-step latency); phase-C compute overlaps
            # the phase-B gathers
            # software-pipelined: emit group g+1's gather/aggregation blocks
            # BEFORE group g's GRU so desc-gen and DMA for the next group
            # overlap this group's compute
            groups = [[i for i in (g0, g0 + 1) if i < NNT]
                      for g0 in range(0, NNT, 2)]

            def emit_group_blocks(tiles):
                for i in tiles:
                    for b in range(4 * i, min(4 * i + 4, NBLK)):
                        emit_block(b)

            emit_group_blocks(groups[0])
            for gi, tiles in enumerate(groups):
                if gi + 1 < len(groups):
                    emit_group_blocks(groups[gi + 1])
                sts = [tile_state(i) for i in tiles]
                for t in range(T):
                    for st in sts:
                        emit_step(st, t)
                for st in sts:
                    emit_tail(st)

    nc.finalize()
    return nc


def kernel(**inputs):
    from concourse import bass_utils

    x = np.asarray(inputs["x"], np.float32)
    dm_arr, wv_arr, ix_arr, x0, x1, KLO, KHI, CK, ck0 = _host_prep(
        x, np.asarray(inputs["edge_index"]), np.asarray(inputs["edge_weight"]))
    wts = _fold_weights({k: np.asarray(v) for k, v in inputs.items()})
    totck = int(ck0[-1])

    nc = _build_graph(totck, KLO, KHI, CK, ck0)

    iota = np.broadcast_to(np.arange(P, dtype=np.float16), (P, P)).copy()
    ident = np.eye(P, dtype=np.float16)
    ones16 = np.ones((16, P), np.float16)
    sel12 = np.zeros((T, T * P), np.float16)
    for t in range(T):
        sel12[t, t * P:(t + 1) * P] = 1.0
    shared = dict(x0=x0, x1=x1, iota=iota, ident=ident, ones16=ones16,
                  sel12=sel12, **wts)
    in_maps = [dict(ix=ix_arr[c], dm=dm_arr[c], wv=wv_arr[c], **shared)
               for c in range(NCORE)]

    res = bass_utils.run_bass_kernel_spmd(
        nc, in_maps, core_ids=list(range(NCORE)))
    kernel._last_results = res
    out = np.concatenate(
        [np.asarray(res.results[c]["out"]).T[:PERCORE] for c in range(NCORE)])
    return np.ascontiguousarray(out, dtype=np.float32)
